# revision 1
# baseline (speedup 1.0000x reference)
import dataclasses
import os
import sys

import numpy as np

# ---------------- numpy reference implementation (fallback path) -------------
_m = np.zeros((7, 7), np.float32)
_m[:3, :] = 1.0
_m[3, :3] = 1.0
MASK_A = _m

LAST_EXEC_NS = None


def _sigmoid(x):
    return 1.0 / (1.0 + np.exp(-x))


def _masked_conv7(x, w, b):
    B, _, H, W = x.shape
    wm = (w * MASK_A[None, None]).astype(np.float32)
    xp = np.pad(x, ((0, 0), (0, 0), (3, 3), (3, 3)))
    out = np.zeros((B, 16, H, W), np.float32)
    for di in range(7):
        for dj in range(7):
            wt = wm[:, 0, di, dj]
            if not np.any(wt):
                continue
            patch = xp[:, 0, di:di + H, dj:dj + W]
            out += wt[None, :, None, None] * patch[:, None]
    return out + b[None, :, None, None]


def _skew(x):
    B, C, H, W = x.shape
    out = np.zeros((B, C, H, H + W - 1), x.dtype)
    for i in range(H):
        out[:, :, i, i:i + W] = x[:, :, i, :]
    return out


def _unskew(xs, W):
    B, C, H, _ = xs.shape
    out = np.empty((B, C, H, W), xs.dtype)
    for i in range(H):
        out[:, :, i, :] = xs[:, :, i, i:i + W]
    return out


def _diag_lstm(x, wi, bi, ws0, ws1):
    B, C, H, W = x.shape
    xs = _skew(x)
    T = H + W - 1
    i2s = np.einsum('oc,bcht->boht', wi, xs) + bi[None, :, None, None]
    h = np.zeros((B, C, H), np.float32)
    c = np.zeros((B, C, H), np.float32)
    hs = np.empty((B, C, H, T), np.float32)
    for t in range(T):
        col = i2s[:, :, :, t]
        h_up = np.concatenate([np.zeros((B, C, 1), np.float32), h[:, :, :-1]], axis=2)
        s = col + np.einsum('oc,bch->boh', ws1, h) + np.einsum('oc,bch->boh', ws0, h_up)
        o, f, i, g = np.split(s, 4, axis=1)
        c = _sigmoid(f) * c + _sigmoid(i) * np.tanh(g)
        h = _sigmoid(o) * np.tanh(c)
        hs[:, :, :, t] = h
    return _unskew(hs, W)


def _bilstm_layer(x, wi2, bi2, w02, w12):
    fwd = _diag_lstm(x, wi2[0], bi2[0], w02[0], w12[0])
    xr = x[:, :, :, ::-1]
    bwd = _diag_lstm(xr, wi2[1], bi2[1], w02[1], w12[1])[:, :, :, ::-1]
    bwd = np.concatenate([np.zeros_like(bwd[:, :, :1]), bwd[:, :, :-1]], axis=2)
    return x + fwd + bwd


def _pixelrnn_np(X, conv_in_w, conv_in_b, rnn_i2s_w, rnn_i2s_b, rnn_ss_w0,
                 rnn_ss_w1, out_w1, out_b1, out_w2, out_b2, out_w3, out_b3):
    h = _masked_conv7(X, conv_in_w, conv_in_b)
    for l in range(7):
        h = _bilstm_layer(h, rnn_i2s_w[l], rnn_i2s_b[l], rnn_ss_w0[l], rnn_ss_w1[l])
    for w, b in ((out_w1, out_b1), (out_w2, out_b2)):
        h = np.maximum(np.einsum('oc,bchw->bohw', w, h) + b[None, :, None, None], 0.0)
    return np.einsum('oc,bchw->bohw', out_w3, h) + out_b3[None, :, None, None]


# =========================== Bass fast path ==================================
# 8-core pure data parallel (4 samples/core). Per layer the fwd+bwd diagonal
# scans run stacked in one pipeline: state stack S [64p, b=4, y'=65, t=128]
# (x_f 0:16 / x_b 16:32 / h 32:64, y'=0 is a zero guard row). Per scan step:
# two accumulating matmuls (K=64 on x+h, K=32 on y-shifted h = h_up), gate
# sigmoid/tanh on ACT with fused per-channel bias, cell/hidden update on DVE,
# h written straight back into S at column t+1. Residuals and the output head
# run in a folded [128=(whi,c), 2048=(b,y,wlo)] layout via DMA reshapes.

B_LOC, H, W, T = 4, 64, 64, 127
NL = 7


def _prep_arrays(inputs):
    import ml_dtypes
    bf16 = ml_dtypes.bfloat16
    X = np.asarray(inputs["X"], np.float32)
    wm = (np.asarray(inputs["conv_in_w"], np.float32) * MASK_A[None, None])
    taps = [(dy, dx) for dy in range(7) for dx in range(7) if MASK_A[dy, dx]]
    convw = np.zeros((len(taps), 16), np.float32)
    for k, (dy, dx) in enumerate(taps):
        convw[k, :] = wm[:, 0, dy, dx]
    convb = np.asarray(inputs["conv_in_b"], np.float32).reshape(16, 1)

    i2s_w = np.asarray(inputs["rnn_i2s_w"], np.float32)
    i2s_b = np.asarray(inputs["rnn_i2s_b"], np.float32)
    ss_w0 = np.asarray(inputs["rnn_ss_w0"], np.float32)
    ss_w1 = np.asarray(inputs["rnn_ss_w1"], np.float32)

    wA = np.zeros((64, NL, 128), np.float32)
    wB = np.zeros((32, NL, 128), np.float32)
    biasT = np.zeros((128, NL), np.float32)
    for l in range(NL):
        for d in range(2):
            wi, w0, w1 = i2s_w[l, d], ss_w0[l, d], ss_w1[l, d]
            for g in range(4):
                cols = slice(g * 32 + d * 16, g * 32 + d * 16 + 16)
                wA[d * 16:(d + 1) * 16, l, cols] = wi[g * 16:(g + 1) * 16, :].T
                wA[32 + d * 16:32 + (d + 1) * 16, l, cols] = w1[g * 16:(g + 1) * 16, :].T
                wB[d * 16:(d + 1) * 16, l, cols] = w0[g * 16:(g + 1) * 16, :].T
                biasT[cols, l] = i2s_b[l, d, g * 16:(g + 1) * 16]

    w1_, w2_ = np.asarray(inputs["out_w1"], np.float32), np.asarray(inputs["out_w2"], np.float32)
    w3_ = np.asarray(inputs["out_w3"], np.float32)
    hw1 = np.zeros((128, 128), np.float32)
    hw2 = np.zeros((128, 128), np.float32)
    hw3 = np.zeros((128, 8), np.float32)
    for whi in range(8):
        s0 = whi * 16
        hw1[s0:s0 + 16, s0:s0 + 16] = w1_.T
        hw2[s0:s0 + 16, s0:s0 + 16] = w2_.T
        hw3[s0:s0 + 16, whi] = w3_[0, :]
    # partition p = whi*16 + oc  ->  bias = out_b1[oc]
    hb1 = np.zeros((128, 1), np.float32)
    hb2 = np.zeros((128, 1), np.float32)
    for whi in range(8):
        hb1[whi * 16:whi * 16 + 16, 0] = np.asarray(inputs["out_b1"], np.float32)
        hb2[whi * 16:whi * 16 + 16, 0] = np.asarray(inputs["out_b2"], np.float32)
    hb3 = np.full((8, 1), np.asarray(inputs["out_b3"], np.float32)[0], np.float32)

    shared = {
        "convw": convw.astype(bf16), "convb": convb,
        "wA": wA.astype(bf16), "wB": wB.astype(bf16), "biasT": biasT,
        "hw1": hw1.astype(bf16), "hw2": hw2.astype(bf16), "hw3": hw3.astype(bf16),
        "hb1": hb1, "hb2": hb2, "hb3": hb3,
    }
    in_maps = []
    for c in range(8):
        xs = X[c * B_LOC:(c + 1) * B_LOC, 0]                      # [4,64,64]
        xpad = np.pad(xs, ((0, 0), (3, 3), (3, 3))).astype(bf16)  # [4,70,70]
        m = dict(shared)
        m["xpad"] = xpad
        in_maps.append(m)
    return taps, in_maps


def _build_nc(taps):
    import concourse.bass as bass
    import concourse.mybir as mybir
    from concourse.tile import TileContext
    dt = mybir.dt
    AF = mybir.ActivationFunctionType
    ALU = mybir.AluOpType
    NT = len(taps)

    nc = bass.Bass()
    d_xpad = nc.declare_dram_parameter("xpad", [B_LOC, 70, 70], dt.bfloat16, isOutput=False)
    d_convw = nc.declare_dram_parameter("convw", [NT, 16], dt.bfloat16, isOutput=False)
    d_convb = nc.declare_dram_parameter("convb", [16, 1], dt.float32, isOutput=False)
    d_wA = nc.declare_dram_parameter("wA", [64, NL, 128], dt.bfloat16, isOutput=False)
    d_wB = nc.declare_dram_parameter("wB", [32, NL, 128], dt.bfloat16, isOutput=False)
    d_biasT = nc.declare_dram_parameter("biasT", [128, NL], dt.float32, isOutput=False)
    d_hw1 = nc.declare_dram_parameter("hw1", [128, 128], dt.bfloat16, isOutput=False)
    d_hw2 = nc.declare_dram_parameter("hw2", [128, 128], dt.bfloat16, isOutput=False)
    d_hw3 = nc.declare_dram_parameter("hw3", [128, 8], dt.bfloat16, isOutput=False)
    d_hb1 = nc.declare_dram_parameter("hb1", [128, 1], dt.float32, isOutput=False)
    d_hb2 = nc.declare_dram_parameter("hb2", [128, 1], dt.float32, isOutput=False)
    d_hb3 = nc.declare_dram_parameter("hb3", [8, 1], dt.float32, isOutput=False)
    d_out = nc.declare_dram_parameter("out", [B_LOC, H, W], dt.float32, isOutput=True)

    def rep(ap, new_ap, offset):
        return dataclasses.replace(ap, ap=[ap.ap[0]] + new_ap, offset=offset)

    def repd(ap, dims, offset):
        return dataclasses.replace(ap, ap=dims, offset=offset)

    with TileContext(nc) as tc:
        with tc.tile_pool(name="persist", bufs=1) as pp:
            S = pp.tile([64, B_LOC, 65, 128], dt.bfloat16)     # state stack
            xres = pp.tile([128, 2304], dt.bfloat16)           # folded residual x, y*36+b*8+wlo
            F = pp.tile([128, 2304], dt.bfloat16)
            Bd = pp.tile([128, 2304], dt.bfloat16)
            wA_t = pp.tile([64, NL, 128], dt.bfloat16)
            wB_t = pp.tile([64, NL, 128], dt.bfloat16)         # rows 32:64 used
            bias_t = pp.tile([128, NL], dt.float32)
            G = pp.tile([128, 256], dt.bfloat16)               # sigmoids o,f,i
            Tt = pp.tile([128, 256], dt.bfloat16)              # tanh(g) at 64:96
            TC = pp.tile([32, 256], dt.bfloat16)               # tanh(c) at 0:32
            Mt = pp.tile([64, 256], dt.bfloat16)               # rows 32:64 used
            Ut = pp.tile([64, 256], dt.bfloat16)               # rows 32:64 used
            Ct = pp.tile([64, 256], dt.bfloat16)               # cell state rows 32:64
            xrev = pp.tile([128, 2304], dt.bfloat16)           # xres with wlo flipped
            junk = pp.tile([32, 64], dt.bfloat16)              # DMA join slivers
            zb = pp.tile([64, 1], dt.float32)                  # zero bias rows 32:64

            nc.sync.dma_start(out=wA_t[:], in_=d_wA[:])
            nc.sync.dma_start(out=wB_t[32:64], in_=d_wB[:])
            nc.sync.dma_start(out=bias_t[:], in_=d_biasT[:])
            nc.gpsimd.memset(S[:], 0.0)
            nc.gpsimd.memset(Bd[:], 0.0)
            nc.gpsimd.memset(xres[:], 0.0)
            nc.gpsimd.memset(F[:], 0.0)
            nc.vector.memset(zb[:], 0.0)

            # ---------------- conv 7x7 masked (layer-0 input) ----------------
            with tc.tile_pool(name="conv", bufs=1) as cp, \
                 tc.tile_pool(name="cpsum", bufs=1, space="PSUM") as cpq:
                P = cp.tile([NT, 16384], dt.bfloat16)
                conv_out = cp.tile([16, 16384], dt.bfloat16)
                convw_t = cp.tile([NT, 16], dt.bfloat16)
                convb_t = cp.tile([16, 1], dt.float32)
                nc.sync.dma_start(out=convw_t[:], in_=d_convw[:])
                nc.sync.dma_start(out=convb_t[:], in_=d_convb[:])
                for k, (dy, dx) in enumerate(taps):
                    src = repd(d_xpad[:, :, :], [[4900, B_LOC], [70, H], [1, W]],
                               dy * 70 + dx)
                    nc.sync.dma_start(out=P[k:k + 1, :], in_=src)
                for j in range((NT + 3) // 4):
                    lo = j * 4
                    hi = min(lo + 4, NT)
                    nc.vector.tensor_copy(junk[0:hi - lo, j:j + 1], P[lo:hi, 0:1])
                for ch in range(8):  # 8 chunks of 2048 cols
                    ps = cpq.tile([16, 2048], dt.float32)
                    for j in range(2):
                        nc.tensor.matmul(out=ps[:, j * 1024:(j + 1) * 1024],
                                         lhsT=convw_t[:],
                                         rhs=P[:, ch * 2048 + j * 1024: ch * 2048 + (j + 1) * 1024],
                                         start=True, stop=True)
                    nc.scalar.activation(conv_out[:, ch * 2048:(ch + 1) * 2048], ps[:],
                                         AF.Identity, bias=convb_t[:], scale=1.0)
                # fold conv_out [16,(b,y,w)] -> xres [128=(whi,c),(y,b,wlo)]
                for whi in range(8):
                    for b in range(B_LOC):
                        src = rep(conv_out[0:16, 0], [[64, H], [1, 8]],
                                  whi * 8 + 4096 * b)
                        dst = rep(xres[whi * 16:whi * 16 + 16, 0],
                                  [[36, H], [1, 8]], 8 * b)
                        nc.sync.dma_start(out=dst, in_=src)
                    sl = rep(xres[whi * 16:whi * 16 + 16, 0], [[8, B_LOC]], 0)
                    nc.vector.tensor_copy(junk[0:16, whi * 4:whi * 4 + 4], sl)

            # ---------------- 7 BiLSTM layers --------------------------------
            with tc.tile_pool(name="spsum", bufs=2, space="PSUM") as pq:
                for l in range(NL):
                    # xrev = xres with wlo reversed within each 8-block
                    nc.vector.tensor_copy(
                        rep(xrev[0:128, 0], [[36, H], [8, B_LOC], [1, 8]], 0),
                        rep(xres[0:128, 0], [[36, H], [8, B_LOC], [-1, 8]], 7))
                    # skew-write x_f / x_b rows of S
                    for whi in range(8):
                        for b in range(B_LOC):
                            src = rep(xres[whi * 16:whi * 16 + 16, 0],
                                      [[36, H], [1, 8]], 8 * b)
                            dstf = rep(S[0:16, 0, 0, 0], [[129, H], [1, 8]],
                                       128 + 8 * whi + 8320 * b)
                            nc.sync.dma_start(out=dstf, in_=src)
                            # x_b: w-reversed; t = y + 56 - 8*whi + wlo'
                            srcb = rep(xrev[whi * 16:whi * 16 + 16, 0],
                                       [[36, H], [1, 8]], 8 * b)
                            dstb = rep(S[16:32, 0, 0, 0], [[129, H], [1, 8]],
                                       128 + 56 - 8 * whi + 8320 * b)
                            nc.sync.dma_start(out=dstb, in_=srcb)
                        slf = rep(S[0:16, 0, 0, 0], [[8320, B_LOC]], 128 + 8 * whi)
                        nc.vector.tensor_copy(junk[0:16, whi * 4:whi * 4 + 4], slf)
                        slb = rep(S[16:32, 0, 0, 0], [[8320, B_LOC]],
                                  128 + 56 - 8 * whi)
                        nc.vector.tensor_copy(junk[16:32, whi * 4:whi * 4 + 4], slb)
                    nc.vector.memset(Ct[32:64, :], 0.0)
                    lhsA = wA_t[:, l, :]
                    lhsB = wB_t[32:64, l, :]
                    b_ofi = bias_t[0:96, l:l + 1]
                    b_g = bias_t[96:128, l:l + 1]
                    for t in range(T):
                        s = pq.tile([128, 256], dt.float32)
                        nc.tensor.matmul(out=s[:], lhsT=lhsA,
                                         rhs=S[0:64, :, 1:65, t],
                                         start=True, stop=False)
                        nc.tensor.matmul(out=s[:], lhsT=lhsB,
                                         rhs=S[32:64, :, 0:64, t],
                                         start=False, stop=True)
                        nc.scalar.activation(G[0:96, :], s[0:96, :], AF.Sigmoid,
                                             bias=b_ofi, scale=1.0)
                        nc.scalar.activation(Tt[64:96, :], s[96:128, :], AF.Tanh,
                                             bias=b_g, scale=1.0)
                        nc.vector.tensor_tensor(Mt[32:64, :], G[64:96, :], Tt[64:96, :], ALU.mult)
                        nc.vector.tensor_tensor(Ut[32:64, :], G[32:64, :], Ct[32:64, :], ALU.mult)
                        nc.vector.tensor_tensor(Ct[32:64, :], Ut[32:64, :], Mt[32:64, :], ALU.add)
                        nc.scalar.activation(TC[0:32, :], Ct[32:64, :], AF.Tanh,
                                             bias=zb[32:64, :], scale=1.0)
                        nc.vector.tensor_tensor(S[32:64, :, 1:65, t + 1], G[0:32, :],
                                                TC[0:32, :], ALU.mult)
                    # residual: unskew fwd/bwd h into folded layout, add
                    for whi in range(8):
                        for b in range(B_LOC):
                            srcf = rep(S[32:48, 0, 0, 0], [[129, H], [1, 8]],
                                       129 + 8 * whi + 8320 * b)
                            dstf2 = rep(F[whi * 16:whi * 16 + 16, 0],
                                        [[36, H], [1, 8]], 8 * b)
                            nc.sync.dma_start(out=dstf2, in_=srcf)
                            # h_b(y-1, w): flat = 129*y + 56 - 8*whi + wlo', y>=1
                            srcb = rep(S[48:64, 0, 0, 0], [[129, H - 1], [1, 8]],
                                       129 + 56 - 8 * whi + 8320 * b)
                            dstb = rep(Bd[whi * 16:whi * 16 + 16, 0],
                                       [[36, H - 1], [1, 8]], 36 + 8 * b)
                            nc.sync.dma_start(out=dstb, in_=srcb)
                        slf = rep(F[whi * 16:whi * 16 + 16, 0], [[8, B_LOC]], 0)
                        nc.vector.tensor_copy(junk[0:16, whi * 4:whi * 4 + 4], slf)
                        slb = rep(Bd[whi * 16:whi * 16 + 16, 0], [[8, B_LOC]], 36)
                        nc.vector.tensor_copy(junk[16:32, whi * 4:whi * 4 + 4], slb)
                    real = [[36, H], [8, B_LOC], [1, 8]]
                    nc.vector.tensor_tensor(
                        rep(xres[0:128, 0], real, 0),
                        rep(xres[0:128, 0], real, 0),
                        rep(F[0:128, 0], real, 0), ALU.add)
                    nc.vector.tensor_tensor(
                        rep(xres[0:128, 0], real, 0),
                        rep(xres[0:128, 0], real, 0),
                        rep(Bd[0:128, 0], [[36, H], [8, B_LOC], [-1, 8]], 7), ALU.add)

            # ---------------- output head ------------------------------------
            with tc.tile_pool(name="head", bufs=1) as hp, \
                 tc.tile_pool(name="hpsum", bufs=2, space="PSUM") as hq:
                hw1_t = hp.tile([128, 128], dt.bfloat16)
                hw2_t = hp.tile([128, 128], dt.bfloat16)
                hw3_t = hp.tile([128, 8], dt.bfloat16)
                hb1_t = hp.tile([128, 1], dt.float32)
                hb2_t = hp.tile([128, 1], dt.float32)
                hb3_t = hp.tile([8, 1], dt.float32)
                h1 = hp.tile([128, 2304], dt.bfloat16)
                h2 = hp.tile([128, 2304], dt.bfloat16)
                outb = hp.tile([8, 2304], dt.float32)
                nc.sync.dma_start(out=hw1_t[:], in_=d_hw1[:])
                nc.sync.dma_start(out=hw2_t[:], in_=d_hw2[:])
                nc.sync.dma_start(out=hw3_t[:], in_=d_hw3[:])
                nc.sync.dma_start(out=hb1_t[:], in_=d_hb1[:])
                nc.sync.dma_start(out=hb2_t[:], in_=d_hb2[:])
                nc.sync.dma_start(out=hb3_t[:], in_=d_hb3[:])
                for srct, wt, bt, dst, M, af in (
                    (xres, hw1_t, hb1_t, h1, 128, AF.Relu),
                    (h1, hw2_t, hb2_t, h2, 128, AF.Relu),
                    (h2, hw3_t, hb3_t, outb, 8, AF.Identity),
                ):
                    for j in range(2):
                        ps = hq.tile([M, 1024], dt.float32)
                        rj = rep(srct[0:128, 0], [[36, 32], [1, 32]], j * 1152)
                        nc.tensor.matmul(out=ps[:], lhsT=wt[:],
                                         rhs=rj, start=True, stop=True)
                        dj = rep(dst[0:M, 0], [[36, 32], [1, 32]], j * 1152)
                        nc.scalar.activation(dj, ps[:], af, bias=bt[:], scale=1.0)
                for b in range(B_LOC):
                    dsto = repd(d_out[:, :, :], [[8, 8], [64, H], [1, 8]], 4096 * b)
                    srco = rep(outb[0:8, 0], [[36, H], [1, 8]], 8 * b)
                    nc.sync.dma_start(out=dsto, in_=srco)
    return nc


def _kernel_bass(**inputs):
    sys.path.insert(0, "/opt/trn_rl_repo")
    from concourse.bass_utils import run_bass_kernel_spmd
    taps, in_maps = _prep_arrays(inputs)
    nc = _build_nc(taps)
    res = run_bass_kernel_spmd(nc, in_maps, list(range(8)))
    global LAST_EXEC_NS
    LAST_EXEC_NS = res.exec_time_ns
    out = np.concatenate([np.asarray(r["out"], np.float32) for r in res.results], axis=0)
    return out.reshape(32, 1, 64, 64)


def kernel(**inputs):
    inputs = {k: np.asarray(v) for k, v in inputs.items()}
    try:
        return _kernel_bass(**inputs)
    except Exception:
        import traceback
        traceback.print_exc()
        return _pixelrnn_np(**inputs).astype(np.float32)

